# revision 13
# baseline (speedup 1.0000x reference)
"""Additive-attention (ContentAttender) Bass kernel for 8 TRN2 NeuronCores.

Problem: B=4, NQ=512, NK=512, D=128, H=32
  kh = keys @ Wk; qh = queries @ Wq
  logits[b,q,k] = w2 . tanh(qh[b,q] + kh[b,k] + b1) + b2
  out = softmax_k(logits) @ keys

Sharding: data-parallel over (batch x query-half) -> 8 cores, each core
handles one batch's 256 queries vs all 512 keys. No collectives.

Per-core pipeline: queries packed 4-per-32-partition-group; the (q,k,h)
tanh tensor is built as 64 broadcast-adds (khT4 + per-group query bias,
split across DVE and GpSimd, bf16) feeding big-chunk ScalarE tanh (the
roofline: ~4.2M tanh elems/core), and the h-contraction with w2 runs on
the TensorEngine via a host-built block-diagonal weight matrix into
32-row PSUM slices. Softmax skips max-subtraction (|logits| <= sum|w2|
~ 3, safe in fp32); b2 dropped (softmax shift-invariant). Normalization
is deferred: context = (exp @ keys) * rowsum^-1.
"""

import contextlib

import numpy as np
import ml_dtypes

import concourse.bass as bass  # noqa: F401
import concourse.mybir as mybir
import concourse.tile as tile
from concourse import bacc
from concourse.bass_utils import run_bass_kernel_spmd

F32 = mybir.dt.float32
BF16 = mybir.dt.bfloat16
AF = mybir.ActivationFunctionType

B, NQ, NK, D, H = 4, 512, 512, 128, 32
NQC = NQ // 2          # queries per core = 256
NG = NQC // 4          # query groups per core = 64

# bundleA1 columns: keysT | WkX4 (Wk tiled 4x along free dim)
KT0, WK0 = 0, 512
NCOLA1 = 640
# bundleA2 columns: queriesT | Wq | b14
QT0, WQ0, B140 = 0, 256, 288
NCOLA2 = 289
# bundleB columns
KX0, W2D0, ID0 = 0, 512, 768
NCOLB = 896

CHUNKS = [4, 8, 16, 16, 14, 4, 2]   # groups per tanh chunk (sum = 64)
TAIL_AFTER = {4: 0, 6: 1}           # chunk index -> softmax bank to emit after

_CACHED_NC = None


def _build_nc():
    nc = bacc.Bacc("TRN2", target_bir_lowering=False, debug=False)

    bundleA1 = nc.declare_dram_parameter(
        "bundleA1", [128, NCOLA1], BF16, isOutput=False
    )
    bundleA2 = nc.declare_dram_parameter(
        "bundleA2", [128, NCOLA2], BF16, isOutput=False
    )
    bundleB = nc.declare_dram_parameter("bundleB", [128, NCOLB], BF16, isOutput=False)
    out = nc.declare_dram_parameter("out", [NQC, D], F32, isOutput=True)

    with tile.TileContext(nc) as tc, contextlib.ExitStack() as ctx:
        cpool = ctx.enter_context(tc.tile_pool(name="consts", bufs=1))
        spool = ctx.enter_context(tc.tile_pool(name="schunk", bufs=2))
        epool = ctx.enter_context(tc.tile_pool(name="softmax", bufs=2))
        ps_kh = ctx.enter_context(tc.tile_pool(name="ps_kh", bufs=1, space="PSUM"))
        ps_qb = ctx.enter_context(tc.tile_pool(name="ps_qb", bufs=1, space="PSUM"))
        ps_logits = ctx.enter_context(
            tc.tile_pool(name="ps_logits", bufs=2, space="PSUM")
        )
        ps_tr = ctx.enter_context(tc.tile_pool(name="ps_tr", bufs=2, space="PSUM"))
        ps_ctx = ctx.enter_context(tc.tile_pool(name="ps_ctx", bufs=2, space="PSUM"))

        bA1 = cpool.tile([128, NCOLA1], BF16, tag="bA1")
        nc.sync.dma_start(bA1[:], bundleA1[:])
        bA2 = cpool.tile([128, NCOLA2], BF16, tag="bA2")
        nc.sync.dma_start(bA2[:], bundleA2[:])
        bB = cpool.tile([128, NCOLB], BF16, tag="bB")
        nc.sync.dma_start(bB[:], bundleB[:])

        kT = bA1[:, KT0 : KT0 + NK]
        WkX4_sb = bA1[:, WK0 : WK0 + 4 * H]
        qT = bA2[:, QT0 : QT0 + NQC]
        Wq_sb = bA2[:, WQ0 : WQ0 + H]
        b14 = bA2[:, B140 : B140 + 1]
        kctx_sb = bB[:, KX0 : KX0 + NK]
        W2D_sb = bB[:, W2D0 : W2D0 + 8 * H]
        id_sb = bB[:, ID0 : ID0 + 128]

        # khT4[(j,h), k] = (keys @ Wk + b1)[k, h] on 4 partition groups:
        # one matmul with Wk tiled 4x along M, b1 folded on copy-out.
        b14f = cpool.tile([128, 1], F32, tag="b14f")
        nc.gpsimd.tensor_copy(b14f[:], b14)
        khT4_ps = ps_kh.tile([128, NK], F32, tag="khps", name="khT4_ps")
        nc.tensor.matmul(khT4_ps[:], WkX4_sb, kT, start=True, stop=True)
        khT4 = cpool.tile([128, NK], BF16, tag="khT4")
        nc.vector.tensor_scalar_add(khT4[:], khT4_ps[:], b14f[:])

        # QB4[(j,h), g] = qh[64j + g, h]
        QB4_ps = ps_qb.tile([128, NG], F32, tag="qbps", name="QB4_ps")
        for j in range(4):
            nc.tensor.matmul(
                QB4_ps[32 * j : 32 * j + 32, :],
                Wq_sb,
                qT[:, NG * j : NG * (j + 1)],
                start=True,
                stop=True,
                tile_position=(0, 32 * j),
            )
        QB4 = cpool.tile([128, NG], F32, tag="QB4")
        nc.vector.tensor_copy(QB4[:], QB4_ps[:])  # gpsimd can't read PSUM

        logits_ps = [None, None]
        g0 = 0

        def emit_chunk(n):
            nonlocal g0
            S = spool.tile([128, max(CHUNKS) * NK], BF16, tag="S", name="S")
            T = spool.tile([128, max(CHUNKS) * NK], BF16, tag="T", name="T")
            for gl in range(n):
                g = g0 + gl
                nc.vector.tensor_scalar_add(
                    S[:, NK * gl : NK * (gl + 1)], khT4[:], QB4[:, g : g + 1]
                )
            nc.scalar.activation(T[:, : NK * n], S[:, : NK * n], AF.Tanh)
            for gl in range(n):
                g = g0 + gl
                beta = g // 32
                s = (g // 8) % 4
                g8 = g % 8
                if logits_ps[beta] is None:
                    logits_ps[beta] = ps_logits.tile(
                        [128, NK], F32, tag="logits", name=f"logits{beta}"
                    )
                nc.tensor.matmul(
                    logits_ps[beta][32 * s : 32 * s + 32, :],
                    W2D_sb[:, 32 * g8 : 32 * g8 + 32],
                    T[:, NK * gl : NK * (gl + 1)],
                    start=(g8 == 0),
                    stop=(g8 == 7),
                    tile_position=(0, 32 * s),
                )
            g0 += n

        def emit_tail(beta):
            E = epool.tile([128, NK], BF16, tag="E", name="E")
            nc.scalar.activation(E[:], logits_ps[beta][:], AF.Exp)
            rs = epool.tile([128, 1], F32, tag="rs", name="rs")
            nc.vector.reduce_sum(rs[:], E[:], axis=mybir.AxisListType.X)
            rr = epool.tile([128, 1], F32, tag="rr", name="rr")
            nc.vector.reciprocal(rr[:], rs[:])
            ET = epool.tile([128, NK], BF16, tag="ET", name="ET")
            for t in range(4):
                trp = ps_tr.tile([128, 128], BF16, tag="tr", name="trp")
                nc.tensor.transpose(trp[:], E[:, 128 * t : 128 * (t + 1)], id_sb)
                nc.vector.tensor_copy(ET[:, 128 * t : 128 * (t + 1)], trp[:])
            ctxp = ps_ctx.tile([128, D], F32, tag="ctx", name="ctxp")
            for t in range(4):
                nc.tensor.matmul(
                    ctxp[:],
                    ET[:, 128 * t : 128 * (t + 1)],
                    kctx_sb[:, 128 * t : 128 * (t + 1)],
                    start=(t == 0),
                    stop=(t == 3),
                )
            ctx_sb = epool.tile([128, D], F32, tag="ctxs", name="ctx_sb")
            nc.vector.tensor_scalar_mul(ctx_sb[:], ctxp[:], rr[:])
            nc.sync.dma_start(out[128 * beta : 128 * (beta + 1), :], ctx_sb[:])

        for ci, n in enumerate(CHUNKS):
            emit_chunk(n)
            if ci in TAIL_AFTER:
                emit_tail(TAIL_AFTER[ci])

    nc.compile()
    return nc


def _get_nc():
    global _CACHED_NC
    if _CACHED_NC is None:
        _CACHED_NC = _build_nc()
    return _CACHED_NC


def _build_w2d(w2):
    """(128, 256): slice g8 has column 4*g8+j = w2 on partitions [32j, 32j+32)."""
    w2d = np.zeros((128, 8 * H), np.float32)
    for g8 in range(8):
        for j in range(4):
            w2d[32 * j : 32 * j + 32, 32 * g8 + 4 * g8 + j] = w2
    return w2d


def _qmap():
    """out row r -> local query index."""
    r = np.arange(NQC)
    beta = r // 128
    p = r % 128
    return 64 * (p % 4) + 32 * beta + 8 * (p // 32) + (p % 32) // 4


def _in_maps(keys, queries, Wk, Wq, b1, w2):
    keys = np.asarray(keys, np.float32)
    queries = np.asarray(queries, np.float32)
    Wk = np.asarray(Wk, np.float32)
    Wq = np.asarray(Wq, np.float32)
    b1 = np.asarray(b1, np.float32)
    w2 = np.asarray(w2, np.float32)

    bundleB = np.zeros((128, NCOLB), np.float32)
    bundleB[:, W2D0 : W2D0 + 8 * H] = _build_w2d(w2)
    bundleB[:, ID0 : ID0 + 128] = np.eye(128, dtype=np.float32)
    b14 = np.tile(b1, 4)  # (128,)

    maps = []
    for c in range(8):
        b, half = divmod(c, 2)
        kb = keys[b]  # (512, 128)
        bA1 = np.zeros((128, NCOLA1), np.float32)
        bA1[:, KT0 : KT0 + NK] = kb.T
        bA1[:, WK0 : WK0 + 4 * H] = np.tile(Wk, (1, 4))
        bA2 = np.zeros((128, NCOLA2), np.float32)
        bA2[:, QT0 : QT0 + NQC] = queries[b, NQC * half : NQC * (half + 1)].T
        bA2[:, WQ0 : WQ0 + H] = Wq
        bA2[:, B140] = b14
        bB = bundleB.copy()
        bB[:, KX0 : KX0 + NK] = (
            kb.reshape(4, 128, 128).transpose(1, 0, 2).reshape(128, 512)
        )
        maps.append(
            {
                "bundleA1": bA1.astype(ml_dtypes.bfloat16),
                "bundleA2": bA2.astype(ml_dtypes.bfloat16),
                "bundleB": bB.astype(ml_dtypes.bfloat16),
            }
        )
    return maps


def _run(in_maps, trace=False):
    nc = _get_nc()
    return run_bass_kernel_spmd(nc, in_maps, core_ids=list(range(8)), trace=trace)


def kernel(keys, queries, Wk, Wq, b1, w2, b2):
    res = _run(_in_maps(keys, queries, Wk, Wq, b1, w2))
    qmap = _qmap()
    outv = np.empty((B, NQ, D), np.float32)
    for c in range(8):
        b, half = divmod(c, 2)
        outv[b, NQC * half + qmap] = res.results[c]["out"]
    return outv
